# revision 4
# baseline (speedup 1.0000x reference)
"""Trainium2 Bass kernel for segment_sum (GAT reduce-sum stage).

out[n, :] = sum over edges e with dst[e] == n of msg[e, :],  n in [0, 50000).

Strategy (8 NeuronCores, SPMD single program):
  - Host routes each edge to the core that owns its destination node
    (node-range sharding: core k owns nodes [k*6250, (k+1)*6250)) and sorts
    each core's edges by destination. Each 128-node tile's edge list is
    padded to a common multiple of 128 (CAP chunks) with zero-message edges.
  - Device, per 128-node tile: for each 128-edge chunk, build the one-hot
    routing matrix onehot[e, n] = (rel_dst[e] == n) with a DVE is_equal
    against a static iota, then accumulate onehot.T @ msg_chunk into PSUM on
    the tensor engine. One [128, 64] fp32 output tile per node tile.
  - No collectives: cores own disjoint node ranges; host concatenates.

msg is shipped as fp16 (exact 0/1 one-hot, fp32 PSUM accumulation), which
halves HBM traffic; relative output error vs the fp32 reference is ~5e-5.
"""

import numpy as np

import concourse.tile as tile
from concourse import bass, mybir
from concourse.bass_utils import run_bass_kernel_spmd
from concourse.vector_clock import ScopedClock

P = 128          # partitions / tile node count / chunk edge count
F = 64           # feature dim
N_CORES = 8
NUM_NODES = 50000
NODES_PER_CORE = NUM_NODES // N_CORES          # 6250
TPC = (NODES_PER_CORE + P - 1) // P            # 49 node tiles per core

_FP16 = mybir.dt.float16
_FP32 = mybir.dt.float32


_MAX_INST_WAITS = 1


def _split_excess_waits(nc, max_waits: int = _MAX_INST_WAITS):
    """This walrus build rejects instructions carrying more than `max_waits`
    sem waits ("Too many sync wait commands"), but Tile's wait pass piles
    every needed proc wait onto the consuming instruction. Hoist the excess
    onto wait-only EventSemaphore instructions inserted just before, on the
    same engine queue (same semantics: queue is in-order)."""
    n = 0
    for f in nc.m.functions:
        for b in f.blocks:
            il = b.instructions
            out = []
            changed = False
            for inst in il:
                si = inst.sync_info
                if si is not None and si.on_wait and len(si.on_wait) > max_waits:
                    waits = list(si.on_wait)
                    extra, keep = waits[:-max_waits], waits[-max_waits:]
                    for i in range(0, len(extra), max_waits):
                        ev = mybir.InstEventSemaphore(
                            name=f"{inst.name}-wsplit{n}",
                            engine=inst.engine,
                            ins=[],
                            outs=[],
                            sync_info=mybir.SyncInfo(
                                on_wait=extra[i:i + max_waits], on_update=[]),
                        )
                        n += 1
                        out.append(ev)
                    inst.sync_info = mybir.SyncInfo(
                        on_wait=keep, on_update=list(si.on_update))
                    changed = True
                out.append(inst)
            if changed:
                b.instructions = out


def _patched_drain_and_barrier(self, tick_clock, wait_clock):
    nc = self.nc
    probe = nc.sync.nop(nofuse=True, hint="drain_waits")
    wait_clock.add_sem_waits(probe.ins, ScopedClock({None: tick_clock.global_clock}))
    si = probe.ins.sync_info
    waits = list(si.on_wait) if si is not None else []
    if si is not None:
        del si.on_wait[:]
    by_name = {h.name: h for h in self.sems.allocated().values()}
    for w in waits:
        assert w.wait_reg is None
        nc.sync.wait_ge(by_name[w.ant_name], w.wait_value)
    nc.sync.drain()

    nc.all_engine_barrier()
    popped = nc._tile_sem_poison_stack.pop()
    assert popped is self._sem_poison
    nc.clear_and_free_semaphores(list(self.sems.allocated().values()))
    nc.all_engine_barrier()

    _split_excess_waits(nc)


tile.TileContext._drain_and_barrier = _patched_drain_and_barrier


def build_program(tpc: int, cap: int, n_cores: int = N_CORES) -> bass.Bass:
    """One SPMD program: [tpc, P, cap*F] fp16 msg + [P, tpc*cap] fp16 rdst
    + [P, P] fp16 iota -> [tpc*P, F] fp32 out."""
    nc = bass.Bass("TRN2", target_bir_lowering=False, debug=False,
                   num_devices=n_cores)
    msg_d = nc.dram_tensor("msg", [tpc, P, cap * F], _FP16,
                           kind="ExternalInput").ap()
    rdst_d = nc.dram_tensor("rdst", [P, tpc * cap], _FP16,
                            kind="ExternalInput").ap()
    iota_d = nc.dram_tensor("iota", [P, P], _FP16, kind="ExternalInput").ap()
    out_d = nc.dram_tensor("out", [tpc * P, F], _FP32,
                           kind="ExternalOutput").ap()

    with tile.TileContext(nc) as tc:
        with (
            tc.tile_pool(name="const", bufs=1) as cpool,
            tc.tile_pool(name="msg", bufs=3) as mpool,
            tc.tile_pool(name="oh", bufs=6) as opool,
            tc.tile_pool(name="outp", bufs=3) as outpool,
            tc.tile_pool(name="psum", bufs=4, space="PSUM") as ppool,
        ):
            iota_t = cpool.tile([P, P], _FP16)
            nc.sync.dma_start(out=iota_t[:], in_=iota_d[:])
            rdst_t = cpool.tile([P, tpc * cap], _FP16)
            nc.sync.dma_start(out=rdst_t[:], in_=rdst_d[:])

            for t in range(tpc):
                mt = mpool.tile([P, cap * F], _FP16)
                nc.sync.dma_start(out=mt[:], in_=msg_d[t])
                ps = ppool.tile([P, F], _FP32)
                for c in range(cap):
                    oh = opool.tile([P, P], _FP16)
                    col = t * cap + c
                    nc.vector.tensor_tensor(
                        out=oh[:],
                        in0=rdst_t[:, col:col + 1].to_broadcast([P, P]),
                        in1=iota_t[:],
                        op=mybir.AluOpType.is_equal,
                    )
                    nc.tensor.matmul(
                        out=ps[:],
                        lhsT=oh[:],
                        rhs=mt[:, c * F:(c + 1) * F],
                        start=(c == 0),
                        stop=(c == cap - 1),
                    )
                ot = outpool.tile([P, F], _FP32)
                nc.scalar.copy(out=ot[:], in_=ps[:])
                nc.sync.dma_start(out=out_d[t * P:(t + 1) * P], in_=ot[:])
    return nc


def prepare_inputs(msg: np.ndarray, edge_index: np.ndarray,
                   num_nodes: int = NUM_NODES, n_cores: int = N_CORES):
    """Route/sort/pad on host. Returns (in_maps, tpc, cap)."""
    E, feat = msg.shape
    assert feat == F
    npc = num_nodes // n_cores
    tpc = (npc + P - 1) // P

    dst = np.asarray(edge_index[1]).astype(np.int64)
    order = np.argsort(dst, kind="stable")
    ds = dst[order]
    n_local = ds % npc
    gtile = (ds // npc) * tpc + n_local // P
    rel = (n_local % P).astype(np.float16)
    counts = np.bincount(gtile, minlength=n_cores * tpc)
    cap = max(1, int(np.ceil(counts.max() / P)))
    S = cap * P

    offs = np.zeros(n_cores * tpc + 1, dtype=np.int64)
    np.cumsum(counts, out=offs[1:])
    pidx = np.full((n_cores * tpc, S), E, dtype=np.int64)
    rds = np.zeros((n_cores * tpc, S), dtype=np.float16)
    for g in range(n_cores * tpc):
        cnt = counts[g]
        pidx[g, :cnt] = order[offs[g]:offs[g] + cnt]
        rds[g, :cnt] = rel[offs[g]:offs[g] + cnt]

    msg16 = np.concatenate(
        [np.asarray(msg, dtype=np.float16), np.zeros((1, F), np.float16)], axis=0)
    gath = msg16[pidx]                                   # [G, S, F]
    msg_dev = np.ascontiguousarray(
        gath.reshape(n_cores, tpc, cap, P, F).transpose(0, 1, 3, 2, 4)
    ).reshape(n_cores, tpc, P, cap * F)
    rdst_dev = np.ascontiguousarray(
        rds.reshape(n_cores, tpc, cap, P).transpose(0, 3, 1, 2)
    ).reshape(n_cores, P, tpc * cap)
    iota = np.ascontiguousarray(
        np.broadcast_to(np.arange(P, dtype=np.float16), (P, P)))

    in_maps = [
        {"msg": msg_dev[k], "rdst": rdst_dev[k], "iota": iota}
        for k in range(n_cores)
    ]
    return in_maps, tpc, cap


def kernel(msg: np.ndarray, edge_index: np.ndarray) -> np.ndarray:
    msg = np.asarray(msg)
    edge_index = np.asarray(edge_index)
    num_nodes = NUM_NODES
    npc = num_nodes // N_CORES

    in_maps, tpc, cap = prepare_inputs(msg, edge_index, num_nodes, N_CORES)
    nc = build_program(tpc, cap, N_CORES)
    res = run_bass_kernel_spmd(nc, in_maps, list(range(N_CORES)))
    out = np.concatenate(
        [res.results[k]["out"][:npc] for k in range(N_CORES)], axis=0)
    return out.astype(np.float32)


# revision 6
# speedup vs baseline: 311.3171x; 311.3171x over previous
"""Trainium2 Bass kernel for segment_sum (GAT reduce-sum stage).

out[n, :] = sum over edges e with dst[e] == n of msg[e, :],  n in [0, 50000).

Strategy (8 NeuronCores, SPMD single program):
  - Host routes each edge to the core that owns its destination node
    (node-range sharding: core k owns nodes [k*6250, (k+1)*6250)) and sorts
    each core's edges by destination. Each 128-node tile's edge list is
    padded to a common multiple of 128 (CAP chunks) with zero-message edges.
  - Device, per 128-node tile: for each 128-edge chunk, build the one-hot
    routing matrix onehot[e, n] = (rel_dst[e] == n) with a DVE is_equal
    against a static iota, then accumulate onehot.T @ msg_chunk into PSUM on
    the tensor engine. One [128, 64] fp32 output tile per node tile.
  - No collectives: cores own disjoint node ranges; host concatenates.

msg is shipped as fp16 (exact 0/1 one-hot, fp32 PSUM accumulation), which
halves HBM traffic; relative output error vs the fp32 reference is ~5e-5.
"""

import numpy as np

import concourse.tile as tile
from concourse import bass, mybir
from concourse.bass_utils import run_bass_kernel_spmd
from concourse.vector_clock import ScopedClock

P = 128          # partitions / tile node count / chunk edge count
F = 64           # feature dim
N_CORES = 8
NUM_NODES = 50000
NODES_PER_CORE = NUM_NODES // N_CORES          # 6250
TPC = (NODES_PER_CORE + P - 1) // P            # 49 node tiles per core

_FP16 = mybir.dt.float16
_FP32 = mybir.dt.float32


_MAX_INST_WAITS = 1


def _split_excess_waits(nc, max_waits: int = _MAX_INST_WAITS):
    """This walrus build rejects instructions carrying more than `max_waits`
    sem waits ("Too many sync wait commands"), but Tile's wait pass piles
    every needed proc wait onto the consuming instruction. Hoist the excess
    onto wait-only EventSemaphore instructions inserted just before, on the
    same engine queue (same semantics: queue is in-order)."""
    n = 0
    for f in nc.m.functions:
        for b in f.blocks:
            il = b.instructions
            out = []
            changed = False
            for inst in il:
                si = inst.sync_info
                if si is not None and si.on_wait and len(si.on_wait) > max_waits:
                    waits = list(si.on_wait)
                    extra, keep = waits[:-max_waits], waits[-max_waits:]
                    for i in range(0, len(extra), max_waits):
                        ev = mybir.InstEventSemaphore(
                            name=f"{inst.name}-wsplit{n}",
                            engine=inst.engine,
                            ins=[],
                            outs=[],
                            sync_info=mybir.SyncInfo(
                                on_wait=extra[i:i + max_waits], on_update=[]),
                        )
                        n += 1
                        out.append(ev)
                    inst.sync_info = mybir.SyncInfo(
                        on_wait=keep, on_update=list(si.on_update))
                    changed = True
                out.append(inst)
            if changed:
                b.instructions = out


def _patched_drain_and_barrier(self, tick_clock, wait_clock):
    nc = self.nc
    probe = nc.sync.nop(nofuse=True, hint="drain_waits")
    wait_clock.add_sem_waits(probe.ins, ScopedClock({None: tick_clock.global_clock}))
    si = probe.ins.sync_info
    waits = list(si.on_wait) if si is not None else []
    if si is not None:
        del si.on_wait[:]
    by_name = {h.name: h for h in self.sems.allocated().values()}
    for w in waits:
        assert w.wait_reg is None
        nc.sync.wait_ge(by_name[w.ant_name], w.wait_value)
    nc.sync.drain()

    nc.all_engine_barrier()
    popped = nc._tile_sem_poison_stack.pop()
    assert popped is self._sem_poison
    nc.clear_and_free_semaphores(list(self.sems.allocated().values()))
    nc.all_engine_barrier()

    _split_excess_waits(nc)


tile.TileContext._drain_and_barrier = _patched_drain_and_barrier


def build_program(tpc: int, cap: int, n_cores: int = N_CORES,
                  repeat: int = 1) -> bass.Bass:
    """One SPMD program: [tpc, P, cap*F] fp16 msg + [P, tpc*cap] fp16 rdst
    + [P, P] fp16 iota -> [tpc*P, F] fp32 out.

    repeat > 1 re-runs the whole body (for steady-state timing via the
    T(N) slope; each repeat writes the same output)."""
    nc = bass.Bass("TRN2", target_bir_lowering=False, debug=False,
                   num_devices=n_cores)
    msg_d = nc.dram_tensor("msg", [tpc, P, cap * F], _FP16,
                           kind="ExternalInput").ap()
    rdst_d = nc.dram_tensor("rdst", [P, tpc * cap], _FP16,
                            kind="ExternalInput").ap()
    iota_d = nc.dram_tensor("iota", [P, P], _FP16, kind="ExternalInput").ap()
    out_d = nc.dram_tensor("out", [tpc * P, F], _FP32,
                           kind="ExternalOutput").ap()

    with tile.TileContext(nc) as tc:
        with (
            tc.tile_pool(name="const", bufs=1) as cpool,
            tc.tile_pool(name="msg", bufs=3) as mpool,
            tc.tile_pool(name="oh", bufs=6) as opool,
            tc.tile_pool(name="outp", bufs=3) as outpool,
            tc.tile_pool(name="psum", bufs=4, space="PSUM") as ppool,
        ):
            iota_t = cpool.tile([P, P], _FP16)
            nc.sync.dma_start(out=iota_t[:], in_=iota_d[:])
            rdst_t = cpool.tile([P, tpc * cap], _FP16)
            nc.sync.dma_start(out=rdst_t[:], in_=rdst_d[:])

            for _r in range(repeat):
                for t in range(tpc):
                    mt = mpool.tile([P, cap * F], _FP16)
                    nc.sync.dma_start(out=mt[:], in_=msg_d[t])
                    ps = ppool.tile([P, F], _FP32)
                    for c in range(cap):
                        oh = opool.tile([P, P], _FP16)
                        col = t * cap + c
                        nc.vector.tensor_tensor(
                            out=oh[:],
                            in0=rdst_t[:, col:col + 1].to_broadcast([P, P]),
                            in1=iota_t[:],
                            op=mybir.AluOpType.is_equal,
                        )
                        nc.tensor.matmul(
                            out=ps[:],
                            lhsT=oh[:],
                            rhs=mt[:, c * F:(c + 1) * F],
                            start=(c == 0),
                            stop=(c == cap - 1),
                        )
                    ot = outpool.tile([P, F], _FP32)
                    nc.scalar.copy(out=ot[:], in_=ps[:])
                    nc.sync.dma_start(out=out_d[t * P:(t + 1) * P], in_=ot[:])
    return nc


def prepare_inputs(msg: np.ndarray, edge_index: np.ndarray,
                   num_nodes: int = NUM_NODES, n_cores: int = N_CORES):
    """Route/sort/pad on host. Returns (in_maps, tpc, cap)."""
    E, feat = msg.shape
    assert feat == F
    npc = num_nodes // n_cores
    tpc = (npc + P - 1) // P

    dst = np.asarray(edge_index[1]).astype(np.int64)
    order = np.argsort(dst, kind="stable")
    ds = dst[order]
    n_local = ds % npc
    gtile = (ds // npc) * tpc + n_local // P
    rel = (n_local % P).astype(np.float16)
    counts = np.bincount(gtile, minlength=n_cores * tpc)
    cap = max(1, int(np.ceil(counts.max() / P)))
    S = cap * P

    offs = np.zeros(n_cores * tpc + 1, dtype=np.int64)
    np.cumsum(counts, out=offs[1:])
    pidx = np.full((n_cores * tpc, S), E, dtype=np.int64)
    rds = np.zeros((n_cores * tpc, S), dtype=np.float16)
    for g in range(n_cores * tpc):
        cnt = counts[g]
        pidx[g, :cnt] = order[offs[g]:offs[g] + cnt]
        rds[g, :cnt] = rel[offs[g]:offs[g] + cnt]

    msg16 = np.concatenate(
        [np.asarray(msg, dtype=np.float16), np.zeros((1, F), np.float16)], axis=0)
    gath = msg16[pidx]                                   # [G, S, F]
    msg_dev = np.ascontiguousarray(
        gath.reshape(n_cores, tpc, cap, P, F).transpose(0, 1, 3, 2, 4)
    ).reshape(n_cores, tpc, P, cap * F)
    rdst_dev = np.ascontiguousarray(
        rds.reshape(n_cores, tpc, cap, P).transpose(0, 3, 1, 2)
    ).reshape(n_cores, P, tpc * cap)
    iota = np.ascontiguousarray(
        np.broadcast_to(np.arange(P, dtype=np.float16), (P, P)))

    in_maps = [
        {"msg": msg_dev[k], "rdst": rdst_dev[k], "iota": iota}
        for k in range(n_cores)
    ]
    return in_maps, tpc, cap


def kernel(msg: np.ndarray, edge_index: np.ndarray) -> np.ndarray:
    msg = np.asarray(msg)
    edge_index = np.asarray(edge_index)
    num_nodes = NUM_NODES
    npc = num_nodes // N_CORES

    in_maps, tpc, cap = prepare_inputs(msg, edge_index, num_nodes, N_CORES)
    nc = build_program(tpc, cap, N_CORES)
    res = run_bass_kernel_spmd(nc, in_maps, list(range(N_CORES)))
    out = np.concatenate(
        [res.results[k]["out"][:npc] for k in range(N_CORES)], axis=0)
    return out.astype(np.float32)


# revision 18
# speedup vs baseline: 529.8470x; 1.7020x over previous
"""Trainium2 Bass kernel for segment_sum (GAT reduce-sum stage).

out[n, :] = sum over edges e with dst[e] == n of msg[e, :],  n in [0, 50000).

Strategy (8 NeuronCores, SPMD single program):
  - Host routes each edge to the core that owns its destination node
    (node-range sharding: core k owns nodes [k*6250, (k+1)*6250)) and sorts
    each core's edges by destination. Each 128-node tile's edge list is
    padded to a common multiple of 128 (CAP chunks) with zero-message edges.
  - Device, per 128-node tile: for each 128-edge chunk, build the one-hot
    routing matrix onehot[e, n] = (rel_dst[e] == n) with a DVE is_equal
    against a static iota, then accumulate onehot.T @ msg_chunk into PSUM on
    the tensor engine. One [128, 64] fp32 output tile per node tile.
  - No collectives: cores own disjoint node ranges; host concatenates.

msg is shipped as fp16 (exact 0/1 one-hot, fp32 PSUM accumulation), which
halves HBM traffic; relative output error vs the fp32 reference is ~5e-5.
"""

import numpy as np

import concourse.tile as tile
from concourse import bass, mybir
from concourse.bass_utils import run_bass_kernel_spmd
from concourse.vector_clock import ScopedClock

P = 128          # partitions / tile node count / chunk edge count
F = 64           # feature dim
N_CORES = 8
NUM_NODES = 50000
NODES_PER_CORE = NUM_NODES // N_CORES          # 6250
TPC = (NODES_PER_CORE + P - 1) // P            # 49 node tiles per core

_FP16 = mybir.dt.float16
_FP32 = mybir.dt.float32


_MAX_INST_WAITS = 1


def _split_excess_waits(nc, max_waits: int = _MAX_INST_WAITS):
    """This walrus build rejects instructions carrying more than `max_waits`
    sem waits ("Too many sync wait commands"), but Tile's wait pass piles
    every needed proc wait onto the consuming instruction. Hoist the excess
    onto wait-only EventSemaphore instructions inserted just before, on the
    same engine queue (same semantics: queue is in-order)."""
    n = 0
    for f in nc.m.functions:
        for b in f.blocks:
            il = b.instructions
            out = []
            changed = False
            for inst in il:
                si = inst.sync_info
                if si is not None and si.on_wait and len(si.on_wait) > max_waits:
                    waits = list(si.on_wait)
                    extra, keep = waits[:-max_waits], waits[-max_waits:]
                    for i in range(0, len(extra), max_waits):
                        ev = mybir.InstEventSemaphore(
                            name=f"{inst.name}-wsplit{n}",
                            engine=inst.engine,
                            ins=[],
                            outs=[],
                            sync_info=mybir.SyncInfo(
                                on_wait=extra[i:i + max_waits], on_update=[]),
                        )
                        n += 1
                        out.append(ev)
                    inst.sync_info = mybir.SyncInfo(
                        on_wait=keep, on_update=list(si.on_update))
                    changed = True
                out.append(inst)
            if changed:
                b.instructions = out


def _patched_drain_and_barrier(self, tick_clock, wait_clock):
    nc = self.nc
    probe = nc.sync.nop(nofuse=True, hint="drain_waits")
    wait_clock.add_sem_waits(probe.ins, ScopedClock({None: tick_clock.global_clock}))
    si = probe.ins.sync_info
    waits = list(si.on_wait) if si is not None else []
    if si is not None:
        del si.on_wait[:]
    by_name = {h.name: h for h in self.sems.allocated().values()}
    for w in waits:
        assert w.wait_reg is None
        nc.sync.wait_ge(by_name[w.ant_name], w.wait_value)
    nc.sync.drain()

    nc.all_engine_barrier()
    popped = nc._tile_sem_poison_stack.pop()
    assert popped is self._sem_poison
    nc.clear_and_free_semaphores(list(self.sems.allocated().values()))
    nc.all_engine_barrier()

    _split_excess_waits(nc)


tile.TileContext._drain_and_barrier = _patched_drain_and_barrier


def build_program(tpc: int, cap: int, n_cores: int = N_CORES,
                  repeat: int = 1) -> bass.Bass:
    """One SPMD program: [tpc, P, cap*F] fp16 msg + [P, tpc*cap] fp16 rdst
    + [P, P] fp16 iota -> [tpc*P, F] fp32 out.

    repeat > 1 re-runs the whole body (for steady-state timing via the
    T(N) slope; each repeat writes the same output)."""
    nc = bass.Bass("TRN2", target_bir_lowering=False, debug=False,
                   num_devices=n_cores)
    msg_d = nc.dram_tensor("msg", [tpc, P, cap * F], _FP16,
                           kind="ExternalInput").ap()
    rdst_d = nc.dram_tensor("rdst", [P, tpc * cap], _FP16,
                            kind="ExternalInput").ap()
    iota_d = nc.dram_tensor("iota", [P, cap * P], _FP16,
                            kind="ExternalInput").ap()
    out_d = nc.dram_tensor("out", [tpc * P, F], _FP32,
                           kind="ExternalOutput").ap()

    with tile.TileContext(nc) as tc:
        with (
            tc.tile_pool(name="const", bufs=1) as cpool,
            tc.tile_pool(name="msg", bufs=3) as mpool,
            tc.tile_pool(name="oh", bufs=6) as opool,
            tc.tile_pool(name="outp", bufs=3) as outpool,
            tc.tile_pool(name="psum", bufs=4, space="PSUM") as ppool,
        ):
            iota_t = cpool.tile([P, cap * P], _FP16)
            nc.sync.dma_start(out=iota_t[:], in_=iota_d[:])
            iota_3d = iota_t[:].rearrange("p (c n) -> p c n", c=cap)
            rdst_t = cpool.tile([P, tpc * cap], _FP16)
            nc.sync.dma_start(out=rdst_t[:], in_=rdst_d[:])

            for _r in range(repeat):
                for t in range(tpc):
                    mt = mpool.tile([P, cap * F], _FP16)
                    nc.sync.dma_start(out=mt[:], in_=msg_d[t])
                    ps = ppool.tile([P, F], _FP32)
                    # one batched is_equal builds all `cap` one-hot chunks
                    oh = opool.tile([P, cap * P], _FP16)
                    nc.vector.tensor_tensor(
                        out=oh[:].rearrange("p (c n) -> p c n", c=cap),
                        in0=rdst_t[:, t * cap:(t + 1) * cap]
                            .to_broadcast([P, cap, P]),
                        in1=iota_3d,
                        op=mybir.AluOpType.is_equal,
                    )
                    for c in range(cap):
                        nc.tensor.matmul(
                            out=ps[:],
                            lhsT=oh[:, c * P:(c + 1) * P],
                            rhs=mt[:, c * F:(c + 1) * F],
                            start=(c == 0),
                            stop=(c == cap - 1),
                        )
                    ot = outpool.tile([P, F], _FP32)
                    nc.scalar.copy(out=ot[:], in_=ps[:])
                    nc.sync.dma_start(out=out_d[t * P:(t + 1) * P], in_=ot[:])
    return nc


def prepare_inputs(msg: np.ndarray, edge_index: np.ndarray,
                   num_nodes: int = NUM_NODES, n_cores: int = N_CORES):
    """Route/sort/pad on host. Returns (in_maps, tpc, cap)."""
    E, feat = msg.shape
    assert feat == F
    npc = num_nodes // n_cores
    tpc = (npc + P - 1) // P

    dst = np.asarray(edge_index[1]).astype(np.int64)
    order = np.argsort(dst, kind="stable")
    ds = dst[order]
    n_local = ds % npc
    gtile = (ds // npc) * tpc + n_local // P
    rel = (n_local % P).astype(np.float16)
    counts = np.bincount(gtile, minlength=n_cores * tpc)
    cap = max(1, int(np.ceil(counts.max() / P)))
    S = cap * P

    offs = np.zeros(n_cores * tpc + 1, dtype=np.int64)
    np.cumsum(counts, out=offs[1:])
    pidx = np.full((n_cores * tpc, S), E, dtype=np.int64)
    rds = np.zeros((n_cores * tpc, S), dtype=np.float16)
    for g in range(n_cores * tpc):
        cnt = counts[g]
        pidx[g, :cnt] = order[offs[g]:offs[g] + cnt]
        rds[g, :cnt] = rel[offs[g]:offs[g] + cnt]

    msg16 = np.concatenate(
        [np.asarray(msg, dtype=np.float16), np.zeros((1, F), np.float16)], axis=0)
    gath = msg16[pidx]                                   # [G, S, F]
    msg_dev = np.ascontiguousarray(
        gath.reshape(n_cores, tpc, cap, P, F).transpose(0, 1, 3, 2, 4)
    ).reshape(n_cores, tpc, P, cap * F)
    rdst_dev = np.ascontiguousarray(
        rds.reshape(n_cores, tpc, cap, P).transpose(0, 3, 1, 2)
    ).reshape(n_cores, P, tpc * cap)
    iota = np.ascontiguousarray(np.broadcast_to(
        np.tile(np.arange(P, dtype=np.float16), cap), (P, cap * P)))

    in_maps = [
        {"msg": msg_dev[k], "rdst": rdst_dev[k], "iota": iota}
        for k in range(n_cores)
    ]
    return in_maps, tpc, cap


# ---------------------------------------------------------------------------
# V2: class-packed matmul reduction (no one-hot building on device).
#
# Each node is assigned a degree class c in {1, 2, 4} holding 16*c edge
# slots (padded with zero messages). Host packs nodes of one class into
# 128-node tiles and lays msg rows out so that each 128-row matmul K-window
# holds exactly 8/c whole nodes. A constant block-ones lhsT [128, 8/c]
# (column m sums slots of node m in the window) reduces a window into 8/c
# PSUM partitions; 16*c matmuls fill a [128, 64] psum tile = 128 node sums.
# PE does all the math; DVE/ACT are idle; the kernel is DMA-bound.
# ---------------------------------------------------------------------------

_CLASSES = (1, 2, 4)  # slots per node = 16*c; nodes per matmul = 8//c


def build_program_v2(tiles_per_class: dict, n_cores: int = N_CORES,
                     repeat: int = 1) -> bass.Bass:
    """tiles_per_class: {c: n_tiles} (same for every core)."""
    nc = bass.Bass("TRN2", target_bir_lowering=False, debug=False,
                   num_devices=n_cores)
    seq = [c for c in _CLASSES for _ in range(tiles_per_class.get(c, 0))]
    total_rows = sum(128 * 16 * c for c in seq)
    n_tiles = len(seq)
    # lhsT_j for matmul j is a shifted block pattern; all 16*c variants are
    # 128-wide sliding windows of one [128, 256] tile per class:
    #   V_c[k, 128 + q] = 1 iff q == k // (16*c);  lhsT_j = V_c[:, 128-j*npm:]
    lt_w = len(_CLASSES) * 2 * P
    msg_d = nc.dram_tensor("msg", [total_rows, F], _FP16,
                           kind="ExternalInput").ap()
    lt_d = nc.dram_tensor("lt", [P, lt_w], _FP16, kind="ExternalInput").ap()
    out_d = nc.dram_tensor("out", [n_tiles * P, F], _FP32,
                           kind="ExternalOutput").ap()

    lt_off = {c: i * 2 * P for i, c in enumerate(_CLASSES)}

    with tile.TileContext(nc) as tc:
        with (
            tc.tile_pool(name="const", bufs=1) as cpool,
            tc.tile_pool(name="msg", bufs=3) as mpool,
            tc.tile_pool(name="outp", bufs=3) as outpool,
            tc.tile_pool(name="psum", bufs=4, space="PSUM") as ppool,
        ):
            lt_t = cpool.tile([P, lt_w], _FP16)
            nc.sync.dma_start(out=lt_t[:], in_=lt_d[:])

            for _r in range(repeat):
                row = 0
                for t, c in enumerate(seq):
                    n_mm = 16 * c            # matmuls per tile
                    mt = mpool.tile([P, n_mm * F], _FP16, tag=f"mt{c}")
                    nc.sync.dma_start(
                        out=mt[:],
                        in_=msg_d[row:row + 128 * n_mm]
                            .rearrange("(p j) f -> p (j f)", p=P))
                    row += 128 * n_mm
                    ps = ppool.tile([P, F], _FP32)
                    npm = 8 // c
                    for j in range(n_mm):
                        o = lt_off[c] + P - j * npm
                        nc.tensor.matmul(
                            out=ps[:],
                            lhsT=lt_t[:, o:o + P],
                            rhs=mt[:, j * F:(j + 1) * F],
                            start=(j == 0), stop=(j == n_mm - 1),
                        )
                    ot = outpool.tile([P, F], _FP32)
                    nc.scalar.copy(out=ot[:], in_=ps[:])
                    nc.sync.dma_start(out=out_d[t * P:(t + 1) * P], in_=ot[:])
    return nc


def prepare_inputs_v2(msg: np.ndarray, edge_index: np.ndarray,
                      num_nodes: int = NUM_NODES, n_cores: int = N_CORES):
    """Returns (in_maps, tiles_per_class, perm) where perm[k] maps the k-th
    core's output rows (class-sorted node order) to local node ids."""
    E, feat = msg.shape
    assert feat == F
    npc = num_nodes // n_cores

    dst = np.asarray(edge_index[1]).astype(np.int64)
    order = np.argsort(dst, kind="stable")     # edges sorted by dst
    ds = dst[order]
    deg = np.bincount(dst, minlength=num_nodes)
    max_deg = int(deg.max())
    assert max_deg <= 64, f"degree {max_deg} exceeds class capacity"
    # class per node: 16*c slots, c in {1,2,4}
    cls = np.full(num_nodes, 4, dtype=np.int64)
    cls[deg <= 32] = 2
    cls[deg <= 16] = 1

    # per-core per-class node counts -> global tile structure
    core_of = np.arange(num_nodes) // npc
    tiles_per_class = {}
    counts = np.zeros((n_cores, len(_CLASSES)), dtype=np.int64)
    for i, c in enumerate(_CLASSES):
        counts[:, i] = np.bincount(core_of[cls == c], minlength=n_cores)
    for i, c in enumerate(_CLASSES):
        tiles_per_class[c] = int(np.ceil(counts[:, i].max() / P))

    # edge start offset per node (into `order`)
    starts = np.zeros(num_nodes + 1, dtype=np.int64)
    np.cumsum(deg, out=starts[1:])

    msg16 = np.concatenate(
        [np.asarray(msg, dtype=np.float16), np.zeros((1, F), np.float16)],
        axis=0)

    in_maps = []
    perms = []
    # sliding-window lhsT patterns: V_c[k, 128+q] = 1 iff q == k//(16*c);
    # matmul j uses the 128-wide window starting at column 128 - j*(8//c)
    lt = np.zeros((P, len(_CLASSES) * 2 * P), dtype=np.float16)
    for i, c in enumerate(_CLASSES):
        k = np.arange(P)
        lt[k, i * 2 * P + P + k // (16 * c)] = 1.0

    for k in range(n_cores):
        lo, hi = k * npc, (k + 1) * npc
        rows_parts = []
        perm_parts = []
        for i, c in enumerate(_CLASSES):
            nodes_c = np.nonzero((core_of == k) & (cls == c))[0]
            cap_nodes = tiles_per_class[c] * P
            S = 16 * c
            # slot index table [cap_nodes, S] -> msg row (E = zero row)
            sidx = np.full((cap_nodes, S), E, dtype=np.int64)
            for j, n in enumerate(nodes_c):
                d = deg[n]
                sidx[j, :d] = order[starts[n]:starts[n] + d]
            # row r of tile grid: tile t2, mm j2, partition p:
            # node idx in tile = j2*(8//c) + p//S, slot = p % S
            sidx = sidx.reshape(tiles_per_class[c], P, S)  # [t2, node, slot]
            npm = 8 // c
            # within tile: mm j2 covers nodes j2*npm..+npm
            # desired layout [t2, p, j2, f]; row (t2, j2, p) = node
            # j2*npm + p//S, slot p%S
            g = sidx.reshape(tiles_per_class[c], 16 * c, npm, S)
            # g[t2, j2, node_in_mm, slot]; K index p = node_in_mm*S + slot
            g = g.reshape(tiles_per_class[c], 16 * c, P)   # [t2, j2, p]
            rows = msg16[g]                                # [t2, j2, P, F]
            rows = np.ascontiguousarray(rows.transpose(0, 2, 1, 3))
            rows_parts.append(rows.reshape(-1, F))
            pp = np.full(cap_nodes, -1, dtype=np.int64)
            pp[:len(nodes_c)] = nodes_c - lo
            perm_parts.append(pp)
        in_maps.append({
            "msg": np.ascontiguousarray(np.concatenate(rows_parts, axis=0)),
            "lt": lt,
        })
        perms.append(np.concatenate(perm_parts))
    return in_maps, tiles_per_class, perms


def kernel_v2(msg: np.ndarray, edge_index: np.ndarray) -> np.ndarray:
    msg = np.asarray(msg)
    edge_index = np.asarray(edge_index)
    npc = NUM_NODES // N_CORES

    in_maps, tiles_per_class, perms = prepare_inputs_v2(
        msg, edge_index, NUM_NODES, N_CORES)
    nc = build_program_v2(tiles_per_class, N_CORES)
    res = run_bass_kernel_spmd(nc, in_maps, list(range(N_CORES)))
    out = np.zeros((NUM_NODES, F), dtype=np.float32)
    for k in range(N_CORES):
        o = res.results[k]["out"]
        valid = perms[k] >= 0
        out[k * npc + perms[k][valid]] = o[valid]
    return out


def kernel(msg: np.ndarray, edge_index: np.ndarray) -> np.ndarray:
    return kernel_v2(msg, edge_index)


def kernel_v1(msg: np.ndarray, edge_index: np.ndarray) -> np.ndarray:
    msg = np.asarray(msg)
    edge_index = np.asarray(edge_index)
    num_nodes = NUM_NODES
    npc = num_nodes // N_CORES

    in_maps, tpc, cap = prepare_inputs(msg, edge_index, num_nodes, N_CORES)
    nc = build_program(tpc, cap, N_CORES)
    res = run_bass_kernel_spmd(nc, in_maps, list(range(N_CORES)))
    out = np.concatenate(
        [res.results[k]["out"][:npc] for k in range(N_CORES)], axis=0)
    return out.astype(np.float32)


# revision 21
# speedup vs baseline: 547.8643x; 1.0340x over previous
"""Trainium2 Bass kernel for segment_sum (GAT reduce-sum stage).

out[n, :] = sum over edges e with dst[e] == n of msg[e, :],  n in [0, 50000).

Strategy (8 NeuronCores, SPMD single program):
  - Host routes each edge to the core that owns its destination node
    (node-range sharding: core k owns nodes [k*6250, (k+1)*6250)) and sorts
    each core's edges by destination. Each 128-node tile's edge list is
    padded to a common multiple of 128 (CAP chunks) with zero-message edges.
  - Device, per 128-node tile: for each 128-edge chunk, build the one-hot
    routing matrix onehot[e, n] = (rel_dst[e] == n) with a DVE is_equal
    against a static iota, then accumulate onehot.T @ msg_chunk into PSUM on
    the tensor engine. One [128, 64] fp32 output tile per node tile.
  - No collectives: cores own disjoint node ranges; host concatenates.

msg is shipped as fp16 (exact 0/1 one-hot, fp32 PSUM accumulation), which
halves HBM traffic; relative output error vs the fp32 reference is ~5e-5.
"""

import numpy as np

import concourse.tile as tile
from concourse import bass, mybir
from concourse.bass_utils import run_bass_kernel_spmd
from concourse.vector_clock import ScopedClock

P = 128          # partitions / tile node count / chunk edge count
F = 64           # feature dim
N_CORES = 8
NUM_NODES = 50000
NODES_PER_CORE = NUM_NODES // N_CORES          # 6250
TPC = (NODES_PER_CORE + P - 1) // P            # 49 node tiles per core

_FP16 = mybir.dt.float16
_FP32 = mybir.dt.float32


_MAX_INST_WAITS = 1


def _split_excess_waits(nc, max_waits: int = _MAX_INST_WAITS):
    """This walrus build rejects instructions carrying more than `max_waits`
    sem waits ("Too many sync wait commands"), but Tile's wait pass piles
    every needed proc wait onto the consuming instruction. Hoist the excess
    onto wait-only EventSemaphore instructions inserted just before, on the
    same engine queue (same semantics: queue is in-order)."""
    n = 0
    for f in nc.m.functions:
        for b in f.blocks:
            il = b.instructions
            out = []
            changed = False
            for inst in il:
                si = inst.sync_info
                if si is not None and si.on_wait and len(si.on_wait) > max_waits:
                    waits = list(si.on_wait)
                    extra, keep = waits[:-max_waits], waits[-max_waits:]
                    for i in range(0, len(extra), max_waits):
                        ev = mybir.InstEventSemaphore(
                            name=f"{inst.name}-wsplit{n}",
                            engine=inst.engine,
                            ins=[],
                            outs=[],
                            sync_info=mybir.SyncInfo(
                                on_wait=extra[i:i + max_waits], on_update=[]),
                        )
                        n += 1
                        out.append(ev)
                    inst.sync_info = mybir.SyncInfo(
                        on_wait=keep, on_update=list(si.on_update))
                    changed = True
                out.append(inst)
            if changed:
                b.instructions = out


def _patched_drain_and_barrier(self, tick_clock, wait_clock):
    nc = self.nc
    probe = nc.sync.nop(nofuse=True, hint="drain_waits")
    wait_clock.add_sem_waits(probe.ins, ScopedClock({None: tick_clock.global_clock}))
    si = probe.ins.sync_info
    waits = list(si.on_wait) if si is not None else []
    if si is not None:
        del si.on_wait[:]
    by_name = {h.name: h for h in self.sems.allocated().values()}
    for w in waits:
        assert w.wait_reg is None
        nc.sync.wait_ge(by_name[w.ant_name], w.wait_value)
    nc.sync.drain()

    nc.all_engine_barrier()
    popped = nc._tile_sem_poison_stack.pop()
    assert popped is self._sem_poison
    nc.clear_and_free_semaphores(list(self.sems.allocated().values()))
    nc.all_engine_barrier()

    _split_excess_waits(nc)


tile.TileContext._drain_and_barrier = _patched_drain_and_barrier


def build_program(tpc: int, cap: int, n_cores: int = N_CORES,
                  repeat: int = 1) -> bass.Bass:
    """One SPMD program: [tpc, P, cap*F] fp16 msg + [P, tpc*cap] fp16 rdst
    + [P, P] fp16 iota -> [tpc*P, F] fp32 out.

    repeat > 1 re-runs the whole body (for steady-state timing via the
    T(N) slope; each repeat writes the same output)."""
    nc = bass.Bass("TRN2", target_bir_lowering=False, debug=False,
                   num_devices=n_cores)
    msg_d = nc.dram_tensor("msg", [tpc, P, cap * F], _FP16,
                           kind="ExternalInput").ap()
    rdst_d = nc.dram_tensor("rdst", [P, tpc * cap], _FP16,
                            kind="ExternalInput").ap()
    iota_d = nc.dram_tensor("iota", [P, cap * P], _FP16,
                            kind="ExternalInput").ap()
    out_d = nc.dram_tensor("out", [tpc * P, F], _FP32,
                           kind="ExternalOutput").ap()

    with tile.TileContext(nc) as tc:
        with (
            tc.tile_pool(name="const", bufs=1) as cpool,
            tc.tile_pool(name="msg", bufs=3) as mpool,
            tc.tile_pool(name="oh", bufs=6) as opool,
            tc.tile_pool(name="outp", bufs=3) as outpool,
            tc.tile_pool(name="psum", bufs=4, space="PSUM") as ppool,
        ):
            iota_t = cpool.tile([P, cap * P], _FP16)
            nc.sync.dma_start(out=iota_t[:], in_=iota_d[:])
            iota_3d = iota_t[:].rearrange("p (c n) -> p c n", c=cap)
            rdst_t = cpool.tile([P, tpc * cap], _FP16)
            nc.sync.dma_start(out=rdst_t[:], in_=rdst_d[:])

            for _r in range(repeat):
                for t in range(tpc):
                    mt = mpool.tile([P, cap * F], _FP16)
                    nc.sync.dma_start(out=mt[:], in_=msg_d[t])
                    ps = ppool.tile([P, F], _FP32)
                    # one batched is_equal builds all `cap` one-hot chunks
                    oh = opool.tile([P, cap * P], _FP16)
                    nc.vector.tensor_tensor(
                        out=oh[:].rearrange("p (c n) -> p c n", c=cap),
                        in0=rdst_t[:, t * cap:(t + 1) * cap]
                            .to_broadcast([P, cap, P]),
                        in1=iota_3d,
                        op=mybir.AluOpType.is_equal,
                    )
                    for c in range(cap):
                        nc.tensor.matmul(
                            out=ps[:],
                            lhsT=oh[:, c * P:(c + 1) * P],
                            rhs=mt[:, c * F:(c + 1) * F],
                            start=(c == 0),
                            stop=(c == cap - 1),
                        )
                    ot = outpool.tile([P, F], _FP32)
                    nc.scalar.copy(out=ot[:], in_=ps[:])
                    nc.sync.dma_start(out=out_d[t * P:(t + 1) * P], in_=ot[:])
    return nc


def prepare_inputs(msg: np.ndarray, edge_index: np.ndarray,
                   num_nodes: int = NUM_NODES, n_cores: int = N_CORES):
    """Route/sort/pad on host. Returns (in_maps, tpc, cap)."""
    E, feat = msg.shape
    assert feat == F
    npc = num_nodes // n_cores
    tpc = (npc + P - 1) // P

    dst = np.asarray(edge_index[1]).astype(np.int64)
    order = np.argsort(dst, kind="stable")
    ds = dst[order]
    n_local = ds % npc
    gtile = (ds // npc) * tpc + n_local // P
    rel = (n_local % P).astype(np.float16)
    counts = np.bincount(gtile, minlength=n_cores * tpc)
    cap = max(1, int(np.ceil(counts.max() / P)))
    S = cap * P

    offs = np.zeros(n_cores * tpc + 1, dtype=np.int64)
    np.cumsum(counts, out=offs[1:])
    pidx = np.full((n_cores * tpc, S), E, dtype=np.int64)
    rds = np.zeros((n_cores * tpc, S), dtype=np.float16)
    for g in range(n_cores * tpc):
        cnt = counts[g]
        pidx[g, :cnt] = order[offs[g]:offs[g] + cnt]
        rds[g, :cnt] = rel[offs[g]:offs[g] + cnt]

    msg16 = np.concatenate(
        [np.asarray(msg, dtype=np.float16), np.zeros((1, F), np.float16)], axis=0)
    gath = msg16[pidx]                                   # [G, S, F]
    msg_dev = np.ascontiguousarray(
        gath.reshape(n_cores, tpc, cap, P, F).transpose(0, 1, 3, 2, 4)
    ).reshape(n_cores, tpc, P, cap * F)
    rdst_dev = np.ascontiguousarray(
        rds.reshape(n_cores, tpc, cap, P).transpose(0, 3, 1, 2)
    ).reshape(n_cores, P, tpc * cap)
    iota = np.ascontiguousarray(np.broadcast_to(
        np.tile(np.arange(P, dtype=np.float16), cap), (P, cap * P)))

    in_maps = [
        {"msg": msg_dev[k], "rdst": rdst_dev[k], "iota": iota}
        for k in range(n_cores)
    ]
    return in_maps, tpc, cap


# ---------------------------------------------------------------------------
# V2: class-packed matmul reduction (no one-hot building on device).
#
# Each node is assigned a degree class s (edge slots, padded with zero
# messages). Host packs nodes of one class into 128-node tiles and lays msg
# rows out so that each matmul K-window (npm = 128//s whole nodes, K =
# npm*s <= 128 rows) is contiguous. The lhsT for window j is a 128-wide
# sliding slice of one static [128, 256] pattern per class (column m sums
# the slots of node m). npm-per-window matmuls accumulate into a [128, 64]
# psum tile = 128 node sums. PE does all the math; DVE/ACT are idle; the
# kernel is DMA-bound.
# ---------------------------------------------------------------------------

_CLASSES = (16, 24, 32, 40, 48, 64)   # slots per node


def _cls_geom(s):
    npm = P // s                 # nodes per matmul window
    mms = -(-P // npm)           # windows per 128-node tile
    kmax = npm * s               # rows per full window
    return npm, mms, kmax


def build_program_v2(tiles_per_class: dict, n_cores: int = N_CORES,
                     repeat: int = 1) -> bass.Bass:
    """tiles_per_class: {c: n_tiles} (same for every core)."""
    nc = bass.Bass("TRN2", target_bir_lowering=False, debug=False,
                   num_devices=n_cores)
    seq = [s for s in _CLASSES for _ in range(tiles_per_class.get(s, 0))]
    total_rows = sum(_cls_geom(s)[1] * _cls_geom(s)[2] for s in seq)
    n_tiles = len(seq)
    # lhsT_j for window j is a shifted block pattern; all variants are
    # 128-wide sliding windows of one [128, 256] tile per class:
    #   V_s[k, 128 + q] = 1 iff q == k // s;  lhsT_j = V_s[:K_j, 128-j*npm:]
    lt_w = len(_CLASSES) * 2 * P
    msg_d = nc.dram_tensor("msg", [total_rows, F], _FP16,
                           kind="ExternalInput").ap()
    lt_d = nc.dram_tensor("lt", [P, lt_w], _FP16, kind="ExternalInput").ap()
    out_d = nc.dram_tensor("out", [n_tiles * P, F], _FP32,
                           kind="ExternalOutput").ap()

    lt_off = {s: i * 2 * P for i, s in enumerate(_CLASSES)}

    with tile.TileContext(nc) as tc:
        with (
            tc.tile_pool(name="const", bufs=1) as cpool,
            tc.tile_pool(name="msg", bufs=3) as mpool,
            tc.tile_pool(name="outp", bufs=3) as outpool,
            tc.tile_pool(name="psum", bufs=4, space="PSUM") as ppool,
        ):
            lt_t = cpool.tile([P, lt_w], _FP16)
            nc.sync.dma_start(out=lt_t[:], in_=lt_d[:])

            for _r in range(repeat):
                row = 0
                for t, s in enumerate(seq):
                    npm, mms, kmax = _cls_geom(s)
                    mt = mpool.tile([P, mms * F], _FP16, tag=f"mt{s}")
                    nc.sync.dma_start(
                        out=mt[:kmax, :],
                        in_=msg_d[row:row + kmax * mms]
                            .rearrange("(p j) f -> p (j f)", p=kmax))
                    row += kmax * mms
                    ps = ppool.tile([P, F], _FP32)
                    for j in range(mms):
                        nodes_j = min(npm, P - j * npm)
                        kj = nodes_j * s
                        o = lt_off[s] + P - j * npm
                        nc.tensor.matmul(
                            out=ps[:],
                            lhsT=lt_t[:kj, o:o + P],
                            rhs=mt[:kj, j * F:(j + 1) * F],
                            start=(j == 0), stop=(j == mms - 1),
                        )
                    ot = outpool.tile([P, F], _FP32)
                    nc.scalar.copy(out=ot[:], in_=ps[:])
                    nc.sync.dma_start(out=out_d[t * P:(t + 1) * P], in_=ot[:])
    return nc


def prepare_inputs_v2(msg: np.ndarray, edge_index: np.ndarray,
                      num_nodes: int = NUM_NODES, n_cores: int = N_CORES):
    """Returns (in_maps, tiles_per_class, perm) where perm[k] maps the k-th
    core's output rows (class-sorted node order) to local node ids."""
    E, feat = msg.shape
    assert feat == F
    npc = num_nodes // n_cores

    dst = np.asarray(edge_index[1]).astype(np.int64)
    order = np.argsort(dst, kind="stable")     # edges sorted by dst
    deg = np.bincount(dst, minlength=num_nodes)
    max_deg = int(deg.max())
    assert max_deg <= max(_CLASSES), \
        f"degree {max_deg} exceeds class capacity {max(_CLASSES)}"
    # class per node: smallest s >= deg
    cls = np.full(num_nodes, max(_CLASSES), dtype=np.int64)
    for s in sorted(_CLASSES, reverse=True):
        cls[deg <= s] = s

    # per-core per-class node counts -> global tile structure
    core_of = np.arange(num_nodes) // npc
    tiles_per_class = {}
    for s in _CLASSES:
        cnt = np.bincount(core_of[cls == s], minlength=n_cores)
        tiles_per_class[s] = int(np.ceil(cnt.max() / P))

    # edge start offset per node (into `order`)
    starts = np.zeros(num_nodes + 1, dtype=np.int64)
    np.cumsum(deg, out=starts[1:])

    msg16 = np.concatenate(
        [np.asarray(msg, dtype=np.float16), np.zeros((1, F), np.float16)],
        axis=0)

    in_maps = []
    perms = []
    # sliding-window lhsT patterns: V_s[k, 128+q] = 1 iff q == k//s;
    # matmul j uses the 128-wide window starting at column 128 - j*npm
    lt = np.zeros((P, len(_CLASSES) * 2 * P), dtype=np.float16)
    for i, s in enumerate(_CLASSES):
        k = np.arange(P)
        lt[k, i * 2 * P + P + k // s] = 1.0

    for k in range(n_cores):
        lo = k * npc
        rows_parts = []
        perm_parts = []
        for s in _CLASSES:
            n_t = tiles_per_class[s]
            if n_t == 0:
                continue
            npm, mms, kmax = _cls_geom(s)
            nodes_c = np.nonzero((core_of == k) & (cls == s))[0]
            cap_nodes = n_t * P
            # slot index table [cap_nodes, s] -> msg row (E = zero row)
            sidx = np.full((cap_nodes, s), E, dtype=np.int64)
            for j, n in enumerate(nodes_c):
                d = deg[n]
                sidx[j, :d] = order[starts[n]:starts[n] + d]
            sidx = sidx.reshape(n_t, P, s)            # [t2, node, slot]
            # grid [t2, p, j]: window j, K-row p -> node j*npm + p//s,
            # slot p%s (invalid -> zero row)
            p = np.arange(kmax)[:, None]
            j = np.arange(mms)[None, :]
            node = j * npm + p // s                   # [kmax, mms]
            slot = np.broadcast_to(p % s, node.shape)
            valid = node < P
            node_c = np.where(valid, node, 0)
            g = sidx[:, node_c, slot]                 # [t2, kmax, mms]
            g = np.where(valid[None], g, E)
            rows = msg16[g]                           # [t2, kmax, mms, F]
            rows_parts.append(rows.reshape(-1, F))
            pp = np.full(cap_nodes, -1, dtype=np.int64)
            pp[:len(nodes_c)] = nodes_c - lo
            perm_parts.append(pp)
        in_maps.append({
            "msg": np.ascontiguousarray(np.concatenate(rows_parts, axis=0)),
            "lt": lt,
        })
        perms.append(np.concatenate(perm_parts))
    return in_maps, tiles_per_class, perms


def kernel_v2(msg: np.ndarray, edge_index: np.ndarray) -> np.ndarray:
    msg = np.asarray(msg)
    edge_index = np.asarray(edge_index)
    npc = NUM_NODES // N_CORES

    in_maps, tiles_per_class, perms = prepare_inputs_v2(
        msg, edge_index, NUM_NODES, N_CORES)
    nc = build_program_v2(tiles_per_class, N_CORES)
    res = run_bass_kernel_spmd(nc, in_maps, list(range(N_CORES)))
    out = np.zeros((NUM_NODES, F), dtype=np.float32)
    for k in range(N_CORES):
        o = res.results[k]["out"]
        valid = perms[k] >= 0
        out[k * npc + perms[k][valid]] = o[valid]
    return out


def kernel(msg: np.ndarray, edge_index: np.ndarray) -> np.ndarray:
    return kernel_v2(msg, edge_index)


def kernel_v1(msg: np.ndarray, edge_index: np.ndarray) -> np.ndarray:
    msg = np.asarray(msg)
    edge_index = np.asarray(edge_index)
    num_nodes = NUM_NODES
    npc = num_nodes // N_CORES

    in_maps, tpc, cap = prepare_inputs(msg, edge_index, num_nodes, N_CORES)
    nc = build_program(tpc, cap, N_CORES)
    res = run_bass_kernel_spmd(nc, in_maps, list(range(N_CORES)))
    out = np.concatenate(
        [res.results[k]["out"][:npc] for k in range(N_CORES)], axis=0)
    return out.astype(np.float32)


# revision 22
# speedup vs baseline: 612.1392x; 1.1173x over previous
"""Trainium2 Bass kernel for segment_sum (GAT reduce-sum stage).

out[n, :] = sum over edges e with dst[e] == n of msg[e, :],  n in [0, 50000).

Strategy (8 NeuronCores, SPMD single program):
  - Host routes each edge to the core that owns its destination node
    (node-range sharding: core k owns nodes [k*6250, (k+1)*6250)) and sorts
    each core's edges by destination. Each 128-node tile's edge list is
    padded to a common multiple of 128 (CAP chunks) with zero-message edges.
  - Device, per 128-node tile: for each 128-edge chunk, build the one-hot
    routing matrix onehot[e, n] = (rel_dst[e] == n) with a DVE is_equal
    against a static iota, then accumulate onehot.T @ msg_chunk into PSUM on
    the tensor engine. One [128, 64] fp32 output tile per node tile.
  - No collectives: cores own disjoint node ranges; host concatenates.

msg is shipped as fp16 (exact 0/1 one-hot, fp32 PSUM accumulation), which
halves HBM traffic; relative output error vs the fp32 reference is ~5e-5.
"""

import numpy as np

import concourse.tile as tile
from concourse import bass, mybir
from concourse.bass_utils import run_bass_kernel_spmd
from concourse.vector_clock import ScopedClock

P = 128          # partitions / tile node count / chunk edge count
F = 64           # feature dim
N_CORES = 8
NUM_NODES = 50000
NODES_PER_CORE = NUM_NODES // N_CORES          # 6250
TPC = (NODES_PER_CORE + P - 1) // P            # 49 node tiles per core

_FP16 = mybir.dt.float16
_FP32 = mybir.dt.float32


_MAX_INST_WAITS = 1


def _split_excess_waits(nc, max_waits: int = _MAX_INST_WAITS):
    """This walrus build rejects instructions carrying more than `max_waits`
    sem waits ("Too many sync wait commands"), but Tile's wait pass piles
    every needed proc wait onto the consuming instruction. Hoist the excess
    onto wait-only EventSemaphore instructions inserted just before, on the
    same engine queue (same semantics: queue is in-order)."""
    n = 0
    for f in nc.m.functions:
        for b in f.blocks:
            il = b.instructions
            out = []
            changed = False
            for inst in il:
                si = inst.sync_info
                if si is not None and si.on_wait and len(si.on_wait) > max_waits:
                    waits = list(si.on_wait)
                    extra, keep = waits[:-max_waits], waits[-max_waits:]
                    for i in range(0, len(extra), max_waits):
                        ev = mybir.InstEventSemaphore(
                            name=f"{inst.name}-wsplit{n}",
                            engine=inst.engine,
                            ins=[],
                            outs=[],
                            sync_info=mybir.SyncInfo(
                                on_wait=extra[i:i + max_waits], on_update=[]),
                        )
                        n += 1
                        out.append(ev)
                    inst.sync_info = mybir.SyncInfo(
                        on_wait=keep, on_update=list(si.on_update))
                    changed = True
                out.append(inst)
            if changed:
                b.instructions = out


def _patched_drain_and_barrier(self, tick_clock, wait_clock):
    nc = self.nc
    probe = nc.sync.nop(nofuse=True, hint="drain_waits")
    wait_clock.add_sem_waits(probe.ins, ScopedClock({None: tick_clock.global_clock}))
    si = probe.ins.sync_info
    waits = list(si.on_wait) if si is not None else []
    if si is not None:
        del si.on_wait[:]
    by_name = {h.name: h for h in self.sems.allocated().values()}
    for w in waits:
        assert w.wait_reg is None
        nc.sync.wait_ge(by_name[w.ant_name], w.wait_value)
    nc.sync.drain()

    nc.all_engine_barrier()
    popped = nc._tile_sem_poison_stack.pop()
    assert popped is self._sem_poison
    nc.clear_and_free_semaphores(list(self.sems.allocated().values()))
    nc.all_engine_barrier()

    _split_excess_waits(nc)


tile.TileContext._drain_and_barrier = _patched_drain_and_barrier


def build_program(tpc: int, cap: int, n_cores: int = N_CORES,
                  repeat: int = 1) -> bass.Bass:
    """One SPMD program: [tpc, P, cap*F] fp16 msg + [P, tpc*cap] fp16 rdst
    + [P, P] fp16 iota -> [tpc*P, F] fp32 out.

    repeat > 1 re-runs the whole body (for steady-state timing via the
    T(N) slope; each repeat writes the same output)."""
    nc = bass.Bass("TRN2", target_bir_lowering=False, debug=False,
                   num_devices=n_cores)
    msg_d = nc.dram_tensor("msg", [tpc, P, cap * F], _FP16,
                           kind="ExternalInput").ap()
    rdst_d = nc.dram_tensor("rdst", [P, tpc * cap], _FP16,
                            kind="ExternalInput").ap()
    iota_d = nc.dram_tensor("iota", [P, cap * P], _FP16,
                            kind="ExternalInput").ap()
    out_d = nc.dram_tensor("out", [tpc * P, F], _FP32,
                           kind="ExternalOutput").ap()

    with tile.TileContext(nc) as tc:
        with (
            tc.tile_pool(name="const", bufs=1) as cpool,
            tc.tile_pool(name="msg", bufs=3) as mpool,
            tc.tile_pool(name="oh", bufs=6) as opool,
            tc.tile_pool(name="outp", bufs=3) as outpool,
            tc.tile_pool(name="psum", bufs=4, space="PSUM") as ppool,
        ):
            iota_t = cpool.tile([P, cap * P], _FP16)
            nc.sync.dma_start(out=iota_t[:], in_=iota_d[:])
            iota_3d = iota_t[:].rearrange("p (c n) -> p c n", c=cap)
            rdst_t = cpool.tile([P, tpc * cap], _FP16)
            nc.sync.dma_start(out=rdst_t[:], in_=rdst_d[:])

            for _r in range(repeat):
                for t in range(tpc):
                    mt = mpool.tile([P, cap * F], _FP16)
                    nc.sync.dma_start(out=mt[:], in_=msg_d[t])
                    ps = ppool.tile([P, F], _FP32)
                    # one batched is_equal builds all `cap` one-hot chunks
                    oh = opool.tile([P, cap * P], _FP16)
                    nc.vector.tensor_tensor(
                        out=oh[:].rearrange("p (c n) -> p c n", c=cap),
                        in0=rdst_t[:, t * cap:(t + 1) * cap]
                            .to_broadcast([P, cap, P]),
                        in1=iota_3d,
                        op=mybir.AluOpType.is_equal,
                    )
                    for c in range(cap):
                        nc.tensor.matmul(
                            out=ps[:],
                            lhsT=oh[:, c * P:(c + 1) * P],
                            rhs=mt[:, c * F:(c + 1) * F],
                            start=(c == 0),
                            stop=(c == cap - 1),
                        )
                    ot = outpool.tile([P, F], _FP32)
                    nc.scalar.copy(out=ot[:], in_=ps[:])
                    nc.sync.dma_start(out=out_d[t * P:(t + 1) * P], in_=ot[:])
    return nc


def prepare_inputs(msg: np.ndarray, edge_index: np.ndarray,
                   num_nodes: int = NUM_NODES, n_cores: int = N_CORES):
    """Route/sort/pad on host. Returns (in_maps, tpc, cap)."""
    E, feat = msg.shape
    assert feat == F
    npc = num_nodes // n_cores
    tpc = (npc + P - 1) // P

    dst = np.asarray(edge_index[1]).astype(np.int64)
    order = np.argsort(dst, kind="stable")
    ds = dst[order]
    n_local = ds % npc
    gtile = (ds // npc) * tpc + n_local // P
    rel = (n_local % P).astype(np.float16)
    counts = np.bincount(gtile, minlength=n_cores * tpc)
    cap = max(1, int(np.ceil(counts.max() / P)))
    S = cap * P

    offs = np.zeros(n_cores * tpc + 1, dtype=np.int64)
    np.cumsum(counts, out=offs[1:])
    pidx = np.full((n_cores * tpc, S), E, dtype=np.int64)
    rds = np.zeros((n_cores * tpc, S), dtype=np.float16)
    for g in range(n_cores * tpc):
        cnt = counts[g]
        pidx[g, :cnt] = order[offs[g]:offs[g] + cnt]
        rds[g, :cnt] = rel[offs[g]:offs[g] + cnt]

    msg16 = np.concatenate(
        [np.asarray(msg, dtype=np.float16), np.zeros((1, F), np.float16)], axis=0)
    gath = msg16[pidx]                                   # [G, S, F]
    msg_dev = np.ascontiguousarray(
        gath.reshape(n_cores, tpc, cap, P, F).transpose(0, 1, 3, 2, 4)
    ).reshape(n_cores, tpc, P, cap * F)
    rdst_dev = np.ascontiguousarray(
        rds.reshape(n_cores, tpc, cap, P).transpose(0, 3, 1, 2)
    ).reshape(n_cores, P, tpc * cap)
    iota = np.ascontiguousarray(np.broadcast_to(
        np.tile(np.arange(P, dtype=np.float16), cap), (P, cap * P)))

    in_maps = [
        {"msg": msg_dev[k], "rdst": rdst_dev[k], "iota": iota}
        for k in range(n_cores)
    ]
    return in_maps, tpc, cap


# ---------------------------------------------------------------------------
# V2: class-packed matmul reduction (no one-hot building on device).
#
# Each node is assigned a degree class s (edge slots, padded with zero
# messages). Host packs nodes of one class into 128-node tiles and lays msg
# rows out so that each matmul K-window (npm = 128//s whole nodes, K =
# npm*s <= 128 rows) is contiguous. The lhsT for window j is a 128-wide
# sliding slice of one static [128, 256] pattern per class (column m sums
# the slots of node m). npm-per-window matmuls accumulate into a [128, 64]
# psum tile = 128 node sums. PE does all the math; DVE/ACT are idle; the
# kernel is DMA-bound.
# ---------------------------------------------------------------------------

_CLASSES = (16, 24, 32, 40, 48, 64)   # slots per node


def _cls_geom(s):
    npm = P // s                 # nodes per matmul window
    mms = -(-P // npm)           # windows per 128-node tile
    kmax = npm * s               # rows per full window
    return npm, mms, kmax


def build_program_v2(tiles_per_class: dict, n_cores: int = N_CORES,
                     repeat: int = 1) -> bass.Bass:
    """tiles_per_class: {c: n_tiles} (same for every core)."""
    nc = bass.Bass("TRN2", target_bir_lowering=False, debug=False,
                   num_devices=n_cores)
    seq = [s for s in _CLASSES for _ in range(tiles_per_class.get(s, 0))]
    total_rows = sum(_cls_geom(s)[1] * _cls_geom(s)[2] for s in seq)
    n_tiles = len(seq)
    # lhsT_j for window j is a shifted block pattern; all variants are
    # 128-wide sliding windows of one [128, 256] tile per class:
    #   V_s[k, 128 + q] = 1 iff q == k // s;  lhsT_j = V_s[:K_j, 128-j*npm:]
    lt_w = len(_CLASSES) * 2 * P
    msg_d = nc.dram_tensor("msg", [total_rows, F], _FP16,
                           kind="ExternalInput").ap()
    lt_d = nc.dram_tensor("lt", [P, lt_w], _FP16, kind="ExternalInput").ap()
    out_d = nc.dram_tensor("out", [n_tiles * P, F], _FP32,
                           kind="ExternalOutput").ap()

    lt_off = {s: i * 2 * P for i, s in enumerate(_CLASSES)}

    with tile.TileContext(nc) as tc:
        with (
            tc.tile_pool(name="const", bufs=1) as cpool,
            tc.tile_pool(name="msg", bufs=3) as mpool,
            tc.tile_pool(name="outp", bufs=3) as outpool,
            tc.tile_pool(name="psum", bufs=4, space="PSUM") as ppool,
        ):
            lt_t = cpool.tile([P, lt_w], _FP16)
            nc.sync.dma_start(out=lt_t[:], in_=lt_d[:])

            for _r in range(repeat):
                row = 0
                for t, s in enumerate(seq):
                    npm, mms, kmax = _cls_geom(s)
                    mt = mpool.tile([P, mms * F], _FP16, tag=f"mt{s}")
                    nc.sync.dma_start(
                        out=mt[:kmax, :],
                        in_=msg_d[row:row + kmax * mms]
                            .rearrange("(p j) f -> p (j f)", p=kmax))
                    row += kmax * mms
                    ps = ppool.tile([P, F], _FP32)
                    for j in range(mms):
                        nodes_j = min(npm, P - j * npm)
                        kj = nodes_j * s
                        o = lt_off[s] + P - j * npm
                        nc.tensor.matmul(
                            out=ps[:],
                            lhsT=lt_t[:kj, o:o + P],
                            rhs=mt[:kj, j * F:(j + 1) * F],
                            start=(j == 0), stop=(j == mms - 1),
                        )
                    ot = outpool.tile([P, F], _FP32)
                    nc.vector.tensor_copy(out=ot[:], in_=ps[:])
                    nc.gpsimd.dma_start(out=out_d[t * P:(t + 1) * P],
                                        in_=ot[:])
    return nc


def prepare_inputs_v2(msg: np.ndarray, edge_index: np.ndarray,
                      num_nodes: int = NUM_NODES, n_cores: int = N_CORES):
    """Returns (in_maps, tiles_per_class, perm) where perm[k] maps the k-th
    core's output rows (class-sorted node order) to local node ids."""
    E, feat = msg.shape
    assert feat == F
    npc = num_nodes // n_cores

    dst = np.asarray(edge_index[1]).astype(np.int64)
    order = np.argsort(dst, kind="stable")     # edges sorted by dst
    deg = np.bincount(dst, minlength=num_nodes)
    max_deg = int(deg.max())
    assert max_deg <= max(_CLASSES), \
        f"degree {max_deg} exceeds class capacity {max(_CLASSES)}"
    # class per node: smallest s >= deg
    cls = np.full(num_nodes, max(_CLASSES), dtype=np.int64)
    for s in sorted(_CLASSES, reverse=True):
        cls[deg <= s] = s

    # per-core per-class node counts -> global tile structure
    core_of = np.arange(num_nodes) // npc
    tiles_per_class = {}
    for s in _CLASSES:
        cnt = np.bincount(core_of[cls == s], minlength=n_cores)
        tiles_per_class[s] = int(np.ceil(cnt.max() / P))

    # edge start offset per node (into `order`)
    starts = np.zeros(num_nodes + 1, dtype=np.int64)
    np.cumsum(deg, out=starts[1:])

    msg16 = np.concatenate(
        [np.asarray(msg, dtype=np.float16), np.zeros((1, F), np.float16)],
        axis=0)

    in_maps = []
    perms = []
    # sliding-window lhsT patterns: V_s[k, 128+q] = 1 iff q == k//s;
    # matmul j uses the 128-wide window starting at column 128 - j*npm
    lt = np.zeros((P, len(_CLASSES) * 2 * P), dtype=np.float16)
    for i, s in enumerate(_CLASSES):
        k = np.arange(P)
        lt[k, i * 2 * P + P + k // s] = 1.0

    for k in range(n_cores):
        lo = k * npc
        rows_parts = []
        perm_parts = []
        for s in _CLASSES:
            n_t = tiles_per_class[s]
            if n_t == 0:
                continue
            npm, mms, kmax = _cls_geom(s)
            nodes_c = np.nonzero((core_of == k) & (cls == s))[0]
            cap_nodes = n_t * P
            # slot index table [cap_nodes, s] -> msg row (E = zero row)
            sidx = np.full((cap_nodes, s), E, dtype=np.int64)
            for j, n in enumerate(nodes_c):
                d = deg[n]
                sidx[j, :d] = order[starts[n]:starts[n] + d]
            sidx = sidx.reshape(n_t, P, s)            # [t2, node, slot]
            # grid [t2, p, j]: window j, K-row p -> node j*npm + p//s,
            # slot p%s (invalid -> zero row)
            p = np.arange(kmax)[:, None]
            j = np.arange(mms)[None, :]
            node = j * npm + p // s                   # [kmax, mms]
            slot = np.broadcast_to(p % s, node.shape)
            valid = node < P
            node_c = np.where(valid, node, 0)
            g = sidx[:, node_c, slot]                 # [t2, kmax, mms]
            g = np.where(valid[None], g, E)
            rows = msg16[g]                           # [t2, kmax, mms, F]
            rows_parts.append(rows.reshape(-1, F))
            pp = np.full(cap_nodes, -1, dtype=np.int64)
            pp[:len(nodes_c)] = nodes_c - lo
            perm_parts.append(pp)
        in_maps.append({
            "msg": np.ascontiguousarray(np.concatenate(rows_parts, axis=0)),
            "lt": lt,
        })
        perms.append(np.concatenate(perm_parts))
    return in_maps, tiles_per_class, perms


def kernel_v2(msg: np.ndarray, edge_index: np.ndarray) -> np.ndarray:
    msg = np.asarray(msg)
    edge_index = np.asarray(edge_index)
    npc = NUM_NODES // N_CORES

    in_maps, tiles_per_class, perms = prepare_inputs_v2(
        msg, edge_index, NUM_NODES, N_CORES)
    nc = build_program_v2(tiles_per_class, N_CORES)
    res = run_bass_kernel_spmd(nc, in_maps, list(range(N_CORES)))
    out = np.zeros((NUM_NODES, F), dtype=np.float32)
    for k in range(N_CORES):
        o = res.results[k]["out"]
        valid = perms[k] >= 0
        out[k * npc + perms[k][valid]] = o[valid]
    return out


def kernel(msg: np.ndarray, edge_index: np.ndarray) -> np.ndarray:
    return kernel_v2(msg, edge_index)


def kernel_v1(msg: np.ndarray, edge_index: np.ndarray) -> np.ndarray:
    msg = np.asarray(msg)
    edge_index = np.asarray(edge_index)
    num_nodes = NUM_NODES
    npc = num_nodes // N_CORES

    in_maps, tpc, cap = prepare_inputs(msg, edge_index, num_nodes, N_CORES)
    nc = build_program(tpc, cap, N_CORES)
    res = run_bass_kernel_spmd(nc, in_maps, list(range(N_CORES)))
    out = np.concatenate(
        [res.results[k]["out"][:npc] for k in range(N_CORES)], axis=0)
    return out.astype(np.float32)
